# revision 10
# baseline (speedup 1.0000x reference)
"""GQA causal attention block (B=2, L=2048, d_model=2048, 32 Q heads / 8 KV heads)
on 8 TRN2 NeuronCores.

Sharding: 8-way tensor parallel over heads, batch-paired. Core c owns q-heads
[4c, 4c+4) and kv-head c FOR BOTH batches (batch 0 lives on SBUF/PE partitions
0:64, batch 1 on 64:128, so the K=64 / M=64 matmuls of the two batches run
concurrently on different quadrants of the PE array).

Each core computes its heads' causal attention for the full sequence in
transposed layout (scores^T = K^T-stationary matmul, so softmax never needs a
P transpose). P~ = exp(S^T) is kept unnormalized in bf16; V is augmented with a
ones column so the AV matmul emits the softmax denominator for free. One 8-core
AllToAll switches head-sharding -> sequence-sharding (attn rows + denominators),
then each core normalizes and runs o_proj against the full Wo for its 512
output rows. No all-reduce; the host just stacks rows.
"""

import os
import sys
import math

os.environ.setdefault("MYCRO_LOCAL_CACHE", "1")
for _p in ("/opt/trn_rl_repo",):
    if os.path.isdir(_p) and _p not in sys.path:
        sys.path.insert(0, _p)

import numpy as np

import concourse.bass as bass
import concourse.bacc as bacc
import concourse.mybir as mybir
import concourse.tile as tile
from concourse.bass_utils import run_bass_kernel_spmd
from concourse.masks import make_identity

F32 = mybir.dt.float32
F32R = mybir.dt.float32r
BF16 = mybir.dt.bfloat16
Exp = mybir.ActivationFunctionType.Exp

D = 2048          # d_model
L = 2048          # sequence length
DH = 64           # head dim
B = 2             # batch
NCORES = 8
NH_L = 4          # local q heads per core (per batch)
QF = NH_L * DH    # 256 local q features per batch
LC1 = 256         # phase-1 l-chunk (moving dim)
NLC1 = L // LC1   # 8
LC = 512          # attention l-tile
NLC = L // LC     # 4
NB = L // 128     # 16 key blocks of 128
SH = QF + NH_L    # 260 rows per A2A shard (4 heads x 64 + 4 denoms)
SCALE = 1.0 / math.sqrt(DH)

_CACHE = {}


def _mmr(nc, out, lhsT, rhs, **kw):
    """fp32 matmul issued as float32r (TF32-ish: full PE rate at N>=256)."""
    nc.tensor.matmul(out, lhsT.bitcast(F32R), rhs.bitcast(F32R), **kw)


def _build_nc():
    nc = bacc.Bacc(
        "TRN2",
        target_bir_lowering=False,
        debug=False,
        enable_asserts=False,
        num_devices=NCORES,
    )
    xT0 = nc.dram_tensor("xT0", [D, L], F32, kind="ExternalInput")
    xT1 = nc.dram_tensor("xT1", [D, L], F32, kind="ExternalInput")
    wqT = nc.dram_tensor("wqT", [D, QF], F32, kind="ExternalInput")
    wkT = nc.dram_tensor("wkT", [D, DH], F32, kind="ExternalInput")
    wvT = nc.dram_tensor("wvT", [D, DH], F32, kind="ExternalInput")
    woT = nc.dram_tensor("woT", [D, D], F32, kind="ExternalInput")
    y = nc.dram_tensor("y", [LC, D], F32, kind="ExternalOutput")

    with tile.TileContext(nc) as tc:
        with tc.tile_pool(name="dram", bufs=1, space="DRAM") as dram:
            bin_ = dram.tile([NCORES * SH, LC], F32, name="bounce_in")
            bout = dram.tile([NCORES * SH, LC], F32, name="bounce_out")
            rdram = dram.tile([32, 512], F32, name="rdram")

            with tc.tile_pool(name="const", bufs=1) as const:
                ident = const.tile([128, 128], F32, name="ident")
                make_identity(nc, ident)

                with tc.tile_pool(name="pers", bufs=1) as pers:
                    # qT tile j: partitions 0:64 = batch0 head j, 64:128 = batch1 head j
                    qT_sb = pers.tile([128, 4 * L], F32R, name="qT_sb")
                    # kT: partitions 0:64 = batch0 kv, 64:128 = batch1 kv
                    kT_sb = pers.tile([128, L], F32R, name="kT_sb")
                    # v_aug block b: cols 0:65 = batch0 (v | ones), 65:130 = batch1
                    vaug = pers.tile([128, NB * 130], BF16, name="vaug")
                    va = vaug.rearrange("p (b c) -> p b c", c=130)
                    nc.gpsimd.memset(va[:, :, 64:65], 1.0)
                    nc.gpsimd.memset(va[:, :, 129:130], 1.0)

                    _phase1_qkv(
                        nc, tc, xT0, xT1, wqT, wkT, wvT, qT_sb, kT_sb, va, ident
                    )
                    _phase2_attn(nc, tc, qT_sb, kT_sb, va, bin_)
                    nc.gpsimd.collective_compute(
                        "AllToAll",
                        mybir.AluOpType.bypass,
                        ins=[bin_.opt()],
                        outs=[bout.opt()],
                        replica_groups=[list(range(NCORES))],
                    )
                    _phase4_oproj(nc, tc, bout, woT, rdram, y)
    nc.finalize()  # bacc: register allocation, ACT table loads, etc.
    return nc


def _phase1_qkv(nc, tc, xT0, xT1, wqT, wkT, wvT, qT_sb, kT_sb, va, ident):
    """qT/kT (feature-major) and v_aug (natural + ones column) projections,
    both batches: batch0 -> output partitions 0:64, batch1 -> 64:128."""
    with (
        tc.tile_pool(name="w1", bufs=1) as wpool,
        tc.tile_pool(name="xc", bufs=2) as xpool,
        tc.tile_pool(name="vt", bufs=2) as vtpool,
        tc.tile_pool(name="p1", bufs=2, space="PSUM") as p1,
    ):
        wq_sb = wpool.tile([128, 16 * QF], F32R, name="wq_sb")
        wk_sb = wpool.tile([128, 16 * DH], F32R, name="wk_sb")
        wv_sb = wpool.tile([128, 16 * DH], F32R, name="wv_sb")
        for w_sb, w_dram, fw in ((wq_sb, wqT, QF), (wk_sb, wkT, DH), (wv_sb, wvT, DH)):
            nc.gpsimd.dma_start(
                w_sb.rearrange("p (b f) -> p b f", f=fw),
                w_dram.rearrange("(b p) f -> p b f", p=128),
            )

        for lc in range(NLC1):
            x0 = xpool.tile([128, 16 * LC1], F32R, name="x0", tag="x0")
            x1 = xpool.tile([128, 16 * LC1], F32R, name="x1", tag="x1")
            for xt, xdram in ((x0, xT0), (x1, xT1)):
                nc.gpsimd.dma_start(
                    xt.rearrange("p (b l) -> p b l", l=LC1),
                    xdram[:, lc * LC1 : (lc + 1) * LC1].rearrange(
                        "(b p) l -> p b l", p=128
                    ),
                )
            # two passes of 3 outputs x 2 batches: (q0, q1, k), then (q2, q3, v).
            # Matmul PSUM dst must start at partition 0 (walrus ISA rule), so
            # batch1 accumulates in its own base-0 psum tile and is routed to
            # SBUF partitions 64:128 via a staging tile + SBUF->SBUF DMA.
            for grp in range(2):
                acc0 = [
                    p1.tile([64, LC1], F32, name=f"a0{i}", tag=f"a0{i}", bufs=1)
                    for i in range(3)
                ]
                acc1 = [
                    p1.tile([64, LC1], F32, name=f"a1{i}", tag=f"a1{i}", bufs=1)
                    for i in range(3)
                ]
                for db in range(16):
                    r0 = x0[:, db * LC1 : (db + 1) * LC1]
                    r1 = x1[:, db * LC1 : (db + 1) * LC1]
                    st = dict(start=(db == 0), stop=(db == 15))
                    for i in range(2):
                        j = 2 * grp + i  # local q head
                        wj = wq_sb[:, db * QF + j * 64 : db * QF + (j + 1) * 64]
                        _mmr(nc, acc0[i][:, :], wj, r0, **st)
                        _mmr(nc, acc1[i][:, :], wj, r1, **st)
                    wkv = wk_sb if grp == 0 else wv_sb
                    wb = wkv[:, db * DH : (db + 1) * DH]
                    _mmr(nc, acc0[2][:, :], wb, r0, **st)
                    _mmr(nc, acc1[2][:, :], wb, r1, **st)
                for i in range(2):
                    j = 2 * grp + i
                    cols = slice(j * L + lc * LC1, j * L + (lc + 1) * LC1)
                    nc.vector.tensor_copy(qT_sb[0:64, cols], acc0[i][:, :])
                    stq = vtpool.tile([64, LC1], F32R, name="stq", tag="stq")
                    nc.vector.tensor_copy(stq[:, :], acc1[i][:, :])
                    nc.sync.dma_start(qT_sb[64:128, cols], stq[:, :])
                if grp == 0:
                    cols = slice(lc * LC1, (lc + 1) * LC1)
                    nc.vector.tensor_copy(kT_sb[0:64, cols], acc0[2][:, :])
                    stk = vtpool.tile([64, LC1], F32R, name="stk", tag="stk")
                    nc.vector.tensor_copy(stk[:, :], acc1[2][:, :])
                    nc.sync.dma_start(kT_sb[64:128, cols], stk[:, :])
                else:
                    vt0 = vtpool.tile([64, LC1], F32, name="vt0", tag="vt0")
                    vt1 = vtpool.tile([64, LC1], F32, name="vt1", tag="vt1")
                    nc.scalar.copy(vt0[:, :], acc0[2][:, :])
                    nc.scalar.copy(vt1[:, :], acc1[2][:, :])
                    for s in range(LC1 // 128):
                        beta = (lc * LC1) // 128 + s
                        tp = p1.tile([128, 128], F32, name="tp", tag="tp")
                        nc.tensor.matmul(
                            tp[:, 0:64],
                            vt0[:, s * 128 : (s + 1) * 128],
                            ident[0:64, 0:64],
                            is_transpose=True,
                        )
                        nc.tensor.matmul(
                            tp[:, 64:128],
                            vt1[:, s * 128 : (s + 1) * 128],
                            ident[0:64, 0:64],
                            is_transpose=True,
                            skip_group_check=True,
                        )
                        nc.scalar.copy(va[:, beta, 0:64], tp[:, 0:64])
                        nc.scalar.copy(va[:, beta, 65:129], tp[:, 64:128])


def _phase2_attn(nc, tc, qT_sb, kT_sb, va, bin_):
    """Transposed-scores causal attention; head j of batch0 on PE rows 0:63,
    head j of batch1 on rows 64:127 (concurrent quadrant matmuls)."""
    with (
        tc.tile_pool(name="p2s", bufs=1, space="PSUM") as scp,
        tc.tile_pool(name="p2o", bufs=2, space="PSUM") as ovp,
        tc.tile_pool(name="pbuf", bufs=1) as pbp,
        tc.tile_pool(name="stg", bufs=3) as stp,
    ):
        for tau in range(NLC):
            for j in range(4):  # local q head
                nb = 4 * tau + 4
                pa = pbp.tile([128, NB * 512], BF16, name="pa", tag="pa")
                pb = pbp.tile([128, NB * 512], BF16, name="pb", tag="pb")
                qa = qT_sb[0:64, j * L + tau * LC : j * L + (tau + 1) * LC]
                qb = qT_sb[64:128, j * L + tau * LC : j * L + (tau + 1) * LC]

                # full (unmasked) strips, two key-blocks per exp call
                for b0 in range(0, 4 * tau, 2):
                    for h, (q, P) in enumerate(((qa, pa), (qb, pb))):
                        po = 64 * h
                        sc = scp.tile([128, 1024], F32, name=f"sc{h}", tag=f"sc{h}")
                        _mmr(
                            nc, sc[:, 0:512],
                            kT_sb[po : po + 64, b0 * 128 : (b0 + 1) * 128], q,
                        )
                        _mmr(
                            nc, sc[:, 512:1024],
                            kT_sb[po : po + 64, (b0 + 1) * 128 : (b0 + 2) * 128], q,
                        )
                        nc.scalar.activation(
                            P[:, b0 * 512 : (b0 + 2) * 512], sc[:, 0:1024], Exp
                        )
                # diagonal strips (block-level causal masking)
                for dj in range(4):
                    beta = 4 * tau + dj
                    for h, (q, P) in enumerate(((qa, pa), (qb, pb))):
                        po = 64 * h
                        sc = scp.tile([128, 1024], F32, name=f"sc{h}", tag=f"sc{h}")
                        _mmr(
                            nc, sc[:, 0:512],
                            kT_sb[po : po + 64, beta * 128 : (beta + 1) * 128], q,
                        )
                        if dj > 0:
                            nc.gpsimd.memset(
                                P[:, beta * 512 : beta * 512 + dj * 128], 0.0
                            )
                        nc.scalar.activation(
                            P[:, beta * 512 + dj * 128 : (beta + 1) * 512],
                            sc[:, dj * 128 : 512],
                            Exp,
                        )
                        dg = P[:, beta * 512 + dj * 128 : beta * 512 + (dj + 1) * 128]
                        # keep where s'_local <= l_local (upper triangular incl diag)
                        nc.gpsimd.affine_select(
                            out=dg,
                            in_=dg,
                            compare_op=mybir.AluOpType.is_ge,
                            fill=0.0,
                            base=0,
                            pattern=[[1, 128]],
                            channel_multiplier=-1,
                        )
                # AV (+denominator via the ones column of v_aug)
                oa = ovp.tile([128, 512], F32, name="oa", tag="oa")
                ob = ovp.tile([128, 512], F32, name="ob", tag="ob")
                for b in range(nb):
                    st = dict(start=(b == 0), stop=(b == nb - 1))
                    nc.tensor.matmul(
                        oa[0:65, :], va[:, b, 0:65],
                        pa[:, b * 512 : (b + 1) * 512], **st,
                    )
                    nc.tensor.matmul(
                        ob[0:65, :], va[:, b, 65:130],
                        pb[:, b * 512 : (b + 1) * 512], **st,
                    )
                # stage attn rows + denominators -> A2A bounce buffer.
                # dest shard for (batch bb, l-block tau) is 4*bb + tau;
                # row inside shard = 64*j (+256..259 for denoms).
                st1 = stp.tile([128, 512], F32, name="st1", tag="st1")
                nc.scalar.copy(st1[0:64, :], oa[0:64, :])
                nc.scalar.copy(st1[64:128, :], ob[0:64, :])
                for bb, half in ((0, st1[0:64, :]), (1, st1[64:128, :])):
                    sh = SH * (4 * bb + tau)
                    nc.sync.dma_start(
                        bin_[sh + 64 * j : sh + 64 * (j + 1), :], half
                    )
                ds = stp.tile([128, 1024], F32, name="ds", tag="ds")
                nc.vector.tensor_copy(ds[64:65, 0:512], oa[64:65, :])
                nc.vector.tensor_copy(ds[64:65, 512:1024], ob[64:65, :])
                for bb in range(2):
                    sh = SH * (4 * bb + tau)
                    nc.sync.dma_start(
                        bin_[sh + QF + j : sh + QF + j + 1, :],
                        ds[64:65, 512 * bb : 512 * bb + 512],
                    )


def _phase4_oproj(nc, tc, bout, woT, rdram, y):
    """Normalize (divide by softmax denominators) and run o_proj for this
    core's 512 sequence rows against the full Wo."""
    with (
        tc.tile_pool(name="an", bufs=1) as anp,
        tc.tile_pool(name="wo", bufs=2) as wop,
        tc.tile_pool(name="den", bufs=1) as denp,
        tc.tile_pool(name="ysb", bufs=2) as yp,
        tc.tile_pool(name="p4y", bufs=4, space="PSUM") as eyp,
    ):
        # denominators: shard c rows 256:260 = heads 4c..4c+3
        dall = denp.tile([32, 512], F32, name="dall")
        for c in range(NCORES):
            nc.sync.dma_start(
                dall[4 * c : 4 * (c + 1), :],
                bout[SH * c + QF : SH * c + QF + NH_L, :],
            )
        rall_f = denp.tile([32, 512], F32, name="rall_f")
        nc.vector.reciprocal(rall_f[:, :], dall[:, :])
        nc.sync.dma_start(rdram[:, :], rall_f[:, :])

        ans = []
        for ft in range(16):
            c, half = divmod(ft, 2)
            au = anp.tile([128, 512], F32, name=f"au{ft}", tag=f"au{ft}")
            nc.sync.dma_start(
                au[:, :],
                bout[SH * c + 128 * half : SH * c + 128 * (half + 1), :],
            )
            hA = 2 * ft
            hB = 2 * ft + 1
            dv = anp.tile([128, 512], F32, name="dv", tag="dv", bufs=2)
            nc.sync.dma_start(dv[0:64, :], rdram[hA : hA + 1, :].partition_broadcast(64))
            nc.sync.dma_start(dv[64:128, :], rdram[hB : hB + 1, :].partition_broadcast(64))
            an = anp.tile([128, 512], F32R, name=f"an{ft}", tag=f"an{ft}")
            nc.vector.tensor_mul(an[:, :], au[:, :], dv[:, :])
            ans.append(an)

        for dc in range(4):
            wo_t = wop.tile([128, 16 * 512], F32R, name="wo_t", tag="wo")
            nc.gpsimd.dma_start(
                wo_t.rearrange("p (b d) -> p b d", d=512),
                woT[:, dc * 512 : (dc + 1) * 512].rearrange("(b p) d -> p b d", p=128),
            )
            for m in range(4):
                yps = eyp.tile([128, 512], F32, name="yps", tag="yps")
                for k in range(16):
                    _mmr(
                        nc, yps[:, :],
                        ans[k][:, m * 128 : (m + 1) * 128],
                        wo_t[:, k * 512 : (k + 1) * 512],
                        start=(k == 0), stop=(k == 15),
                    )
                ysb = yp.tile([128, 512], F32, name="ysb", tag="ysb")
                nc.scalar.copy(ysb[:, :], yps[:, :])
                nc.sync.dma_start(
                    y[m * 128 : (m + 1) * 128, dc * 512 : (dc + 1) * 512], ysb[:, :]
                )


def _get_nc():
    if "nc" not in _CACHE:
        _CACHE["nc"] = _build_nc()
    return _CACHE["nc"]


LAST_EXEC_NS = None


def kernel(x, Wq, Wk, Wv, Wo):
    global LAST_EXEC_NS
    x = np.asarray(x, dtype=np.float32)
    Wq = np.asarray(Wq, dtype=np.float32)
    Wk = np.asarray(Wk, dtype=np.float32)
    Wv = np.asarray(Wv, dtype=np.float32)
    Wo = np.asarray(Wo, dtype=np.float32)

    xT0 = np.ascontiguousarray(x[0].T)
    xT1 = np.ascontiguousarray(x[1].T)
    woT = np.ascontiguousarray(Wo.T)

    in_maps = []
    for c in range(NCORES):
        wqT_c = np.ascontiguousarray((SCALE * Wq[QF * c : QF * (c + 1), :]).T)
        wkT_c = np.ascontiguousarray(Wk[DH * c : DH * (c + 1), :].T)
        wvT_c = np.ascontiguousarray(Wv[DH * c : DH * (c + 1), :].T)
        in_maps.append(
            {
                "xT0": xT0,
                "xT1": xT1,
                "wqT": wqT_c,
                "wkT": wkT_c,
                "wvT": wvT_c,
                "woT": woT,
            }
        )

    nc = _get_nc()
    res = run_bass_kernel_spmd(nc, in_maps, core_ids=list(range(NCORES)))
    LAST_EXEC_NS = getattr(res, "exec_time_ns", None)

    out = np.empty((B, L, D), dtype=np.float32)
    for c in range(NCORES):
        b, g = divmod(c, 4)
        out[b, 512 * g : 512 * (g + 1), :] = res.results[c]["y"]
    return out


# revision 11
# speedup vs baseline: 1.1619x; 1.1619x over previous
"""GQA causal attention block (B=2, L=2048, d_model=2048, 32 Q heads / 8 KV heads)
on 8 TRN2 NeuronCores.

Sharding: 8-way tensor parallel over heads, batch-paired. Core c owns q-heads
[4c, 4c+4) and kv-head c FOR BOTH batches.

Layouts:
  - qT (bf16): 4 tiles [128, L] = head-pair x batch: tile (t, b) holds heads
    2t (partitions 0:64) and 2t+1 (64:128) of batch b, feature-major.
  - kT (bf16): per batch a [128, L] tile with the kv head DUPLICATED in both
    partition halves, so scores for odd heads read lhsT/rhs at matching base 64.
  - v_aug (bf16): per key-block [128, 130]: cols 0:65 = batch0 (v | ones),
    65:130 = batch1 (v | ones).

Per head+batch, causal attention runs in transposed layout: scores^T = matmul
(kT stationary, qT moving), exp on ScalarE straight out of PSUM into bf16 P
tiles (unnormalized), AV matmul against V-with-ones-column emits both attn^T
and the softmax denominator. One 8-core AllToAll switches head-sharding ->
sequence-sharding; each core then normalizes (reciprocal + partition-broadcast
DMA + DVE multiply) and runs o_proj (fp32r) against the full Wo for its 512
output rows. The host just stacks rows.
"""

import os
import sys
import math

os.environ.setdefault("MYCRO_LOCAL_CACHE", "1")
for _p in ("/opt/trn_rl_repo",):
    if os.path.isdir(_p) and _p not in sys.path:
        sys.path.insert(0, _p)

import numpy as np

import concourse.bass as bass
import concourse.bacc as bacc
import concourse.mybir as mybir
import concourse.tile as tile
from concourse.bass_utils import run_bass_kernel_spmd
from concourse.masks import make_identity

F32 = mybir.dt.float32
F32R = mybir.dt.float32r
BF16 = mybir.dt.bfloat16
Exp = mybir.ActivationFunctionType.Exp

D = 2048          # d_model
L = 2048          # sequence length
DH = 64           # head dim
B = 2             # batch
NCORES = 8
NH_L = 4          # local q heads per core (per batch)
QF = NH_L * DH    # 256 local q features per batch
LC1 = 256         # phase-1 l-chunk (moving dim)
NLC1 = L // LC1   # 8
LC = 512          # attention l-tile
NLC = L // LC     # 4
NB = L // 128     # 16 key blocks of 128
SH = QF + NH_L    # 260 rows per A2A shard (4 heads x 64 + 4 denoms)
SCALE = 1.0 / math.sqrt(DH)

_CACHE = {}


def _mmr(nc, out, lhsT, rhs, **kw):
    """float32r matmul (TF32-ish). Operands must come from f32r-producing
    instructions (gpsimd casting DMA / DVE ops)."""
    nc.tensor.matmul(out, lhsT, rhs, **kw)


def _build_nc():
    nc = bacc.Bacc(
        "TRN2",
        target_bir_lowering=False,
        debug=False,
        enable_asserts=False,
        num_devices=NCORES,
    )
    xT0 = nc.dram_tensor("xT0", [D, L], F32, kind="ExternalInput")
    xT1 = nc.dram_tensor("xT1", [D, L], F32, kind="ExternalInput")
    wqT = nc.dram_tensor("wqT", [D, QF], F32, kind="ExternalInput")
    wkT = nc.dram_tensor("wkT", [D, DH], F32, kind="ExternalInput")
    wvT = nc.dram_tensor("wvT", [D, DH], F32, kind="ExternalInput")
    woT = nc.dram_tensor("woT", [D, D], F32, kind="ExternalInput")
    y = nc.dram_tensor("y", [LC, D], F32, kind="ExternalOutput")

    with tile.TileContext(nc) as tc:
        with tc.tile_pool(name="dram", bufs=1, space="DRAM") as dram:
            bin_ = dram.tile([NCORES * SH, LC], F32, name="bounce_in")
            bout = dram.tile([NCORES * SH, LC], F32, name="bounce_out")
            rdram = dram.tile([32, 512], F32, name="rdram")

            with tc.tile_pool(name="const", bufs=1) as const:
                ident = const.tile([128, 128], F32, name="ident")
                make_identity(nc, ident)

                with tc.tile_pool(name="pers", bufs=1) as pers:
                    # q: [pair t][batch b] -> [128, L] bf16 (heads 2t | 2t+1)
                    qT = [
                        [
                            pers.tile([128, L], BF16, name=f"qT{t}{b}")
                            for b in range(2)
                        ]
                        for t in range(2)
                    ]
                    # kT per batch, kv head duplicated in both halves
                    kT = [pers.tile([128, L], BF16, name=f"kT{b}") for b in range(2)]
                    vaug = pers.tile([128, NB * 130], BF16, name="vaug")
                    va = vaug.rearrange("p (b c) -> p b c", c=130)
                    nc.gpsimd.memset(va[:, :, 64:65], 1.0)
                    nc.gpsimd.memset(va[:, :, 129:130], 1.0)

                    _phase1_qkv(nc, tc, xT0, xT1, wqT, wkT, wvT, qT, kT, va, ident)
                    _phase2_attn(nc, tc, qT, kT, va, bin_)
                    nc.gpsimd.collective_compute(
                        "AllToAll",
                        mybir.AluOpType.bypass,
                        ins=[bin_.opt()],
                        outs=[bout.opt()],
                        replica_groups=[list(range(NCORES))],
                    )
                    _phase4_oproj(nc, tc, bout, woT, rdram, y)
    nc.finalize()  # bacc: register allocation, ACT table loads, etc.
    return nc


def _phase1_qkv(nc, tc, xT0, xT1, wqT, wkT, wvT, qT, kT, va, ident):
    """Projections. q: one M=128 fp32r matmul per (head-pair, batch, db).
    k/v: M=64 per batch at psum base 0; the partition-64 halves of kT are
    filled via a bf16 staging tile + SBUF->SBUF DMA (matmul psum dst must
    start at partition 0)."""
    with (
        tc.tile_pool(name="w1", bufs=1) as wpool,
        tc.tile_pool(name="xc", bufs=2) as xpool,
        tc.tile_pool(name="vt", bufs=2) as vtpool,
        tc.tile_pool(name="p1", bufs=1, space="PSUM") as p1,
    ):
        wq_sb = wpool.tile([128, 16 * QF], F32R, name="wq_sb")
        wk_sb = wpool.tile([128, 16 * DH], F32R, name="wk_sb")
        wv_sb = wpool.tile([128, 16 * DH], F32R, name="wv_sb")
        for w_sb, w_dram, fw in ((wq_sb, wqT, QF), (wk_sb, wkT, DH), (wv_sb, wvT, DH)):
            nc.gpsimd.dma_start(
                w_sb.rearrange("p (b f) -> p b f", f=fw),
                w_dram.rearrange("(b p) f -> p b f", p=128),
            )

        for lc in range(NLC1):
            x0 = xpool.tile([128, 16 * LC1], F32R, name="x0", tag="x0")
            x1 = xpool.tile([128, 16 * LC1], F32R, name="x1", tag="x1")
            for xt, xdram in ((x0, xT0), (x1, xT1)):
                nc.gpsimd.dma_start(
                    xt.rearrange("p (b l) -> p b l", l=LC1),
                    xdram[:, lc * LC1 : (lc + 1) * LC1].rearrange(
                        "(b p) l -> p b l", p=128
                    ),
                )
            cols = slice(lc * LC1, (lc + 1) * LC1)
            # pass A: q-pair0 (both batches) + k (both); pass B: q-pair1 + v
            for grp in range(2):
                aq = [
                    p1.tile([128, LC1], F32, name=f"aq{b}", tag=f"aq{b}")
                    for b in range(2)
                ]
                akv = [
                    p1.tile([64, LC1], F32, name=f"akv{b}", tag=f"akv{b}")
                    for b in range(2)
                ]
                for db in range(16):
                    rx = (
                        x0[:, db * LC1 : (db + 1) * LC1],
                        x1[:, db * LC1 : (db + 1) * LC1],
                    )
                    st = dict(start=(db == 0), stop=(db == 15))
                    wjp = wq_sb[:, db * QF + grp * 128 : db * QF + (grp + 1) * 128]
                    wkv = wk_sb if grp == 0 else wv_sb
                    wb = wkv[:, db * DH : (db + 1) * DH]
                    for b in range(2):
                        _mmr(nc, aq[b][:, :], wjp, rx[b], **st)
                        _mmr(nc, akv[b][:, :], wb, rx[b], **st)
                for b in range(2):
                    # q copyback: psum f32 -> bf16, partitions already paired
                    nc.scalar.copy(qT[grp][b][:, cols], aq[b][:, :])
                if grp == 0:
                    for b in range(2):
                        nc.scalar.copy(kT[b][0:64, cols], akv[b][:, :])
                        stk = vtpool.tile([64, LC1], BF16, name="stk", tag=f"stk{b}")
                        nc.scalar.copy(stk[:, :], akv[b][:, :])
                        nc.sync.dma_start(kT[b][64:128, cols], stk[:, :])
                else:
                    vt0 = vtpool.tile([64, LC1], F32, name="vt0", tag="vt0")
                    vt1 = vtpool.tile([64, LC1], F32, name="vt1", tag="vt1")
                    nc.scalar.copy(vt0[:, :], akv[0][:, :])
                    nc.scalar.copy(vt1[:, :], akv[1][:, :])
                    for s in range(LC1 // 128):
                        beta = (lc * LC1) // 128 + s
                        tp = p1.tile([128, 128], F32, name="tp", tag="tp", bufs=2)
                        nc.tensor.matmul(
                            tp[:, 0:64],
                            vt0[:, s * 128 : (s + 1) * 128],
                            ident[0:64, 0:64],
                            is_transpose=True,
                        )
                        nc.tensor.matmul(
                            tp[:, 64:128],
                            vt1[:, s * 128 : (s + 1) * 128],
                            ident[0:64, 0:64],
                            is_transpose=True,
                            skip_group_check=True,
                        )
                        nc.scalar.copy(va[:, beta, 0:64], tp[:, 0:64])
                        nc.scalar.copy(va[:, beta, 65:129], tp[:, 64:128])


def _phase2_attn(nc, tc, qT, kT, va, bin_):
    """Transposed-scores causal attention (bf16 QK^T and AV)."""
    with (
        tc.tile_pool(name="p2s", bufs=1, space="PSUM") as scp,
        tc.tile_pool(name="p2o", bufs=2, space="PSUM") as ovp,
        tc.tile_pool(name="pbuf", bufs=1) as pbp,
        tc.tile_pool(name="stg", bufs=3) as stp,
    ):
        for tau in range(NLC):
            for j in range(4):  # local q head
                t, hh = divmod(j, 2)
                po = 64 * hh  # partition base inside the pair tile
                nb = 4 * tau + 4
                pa = pbp.tile([128, NB * 512], BF16, name="pa", tag="pa")
                pb = pbp.tile([128, NB * 512], BF16, name="pb", tag="pb")
                lcols = slice(tau * LC, (tau + 1) * LC)
                qa = qT[t][0][po : po + 64, lcols]
                qb = qT[t][1][po : po + 64, lcols]

                # full (unmasked) strips, two key-blocks per exp call
                for b0 in range(0, 4 * tau, 2):
                    for hi, (q, kTb, P) in enumerate(
                        ((qa, kT[0], pa), (qb, kT[1], pb))
                    ):
                        sc = scp.tile([128, 1024], F32, name="sc", tag=f"sc{hi}")
                        nc.tensor.matmul(
                            sc[:, 0:512],
                            kTb[po : po + 64, b0 * 128 : (b0 + 1) * 128],
                            q,
                        )
                        nc.tensor.matmul(
                            sc[:, 512:1024],
                            kTb[po : po + 64, (b0 + 1) * 128 : (b0 + 2) * 128],
                            q,
                        )
                        nc.scalar.activation(
                            P[:, b0 * 512 : (b0 + 2) * 512], sc[:, 0:1024], Exp
                        )
                # diagonal strips (block-level causal masking)
                for dj in range(4):
                    beta = 4 * tau + dj
                    for hi, (q, kTb, P) in enumerate(
                        ((qa, kT[0], pa), (qb, kT[1], pb))
                    ):
                        sc = scp.tile([128, 1024], F32, name="sc", tag=f"sc{hi}")
                        nc.tensor.matmul(
                            sc[:, 0:512],
                            kTb[po : po + 64, beta * 128 : (beta + 1) * 128],
                            q,
                        )
                        if dj > 0:
                            nc.gpsimd.memset(
                                P[:, beta * 512 : beta * 512 + dj * 128], 0.0
                            )
                        nc.scalar.activation(
                            P[:, beta * 512 + dj * 128 : (beta + 1) * 512],
                            sc[:, dj * 128 : 512],
                            Exp,
                        )
                        dg = P[:, beta * 512 + dj * 128 : beta * 512 + (dj + 1) * 128]
                        nc.gpsimd.affine_select(
                            out=dg,
                            in_=dg,
                            compare_op=mybir.AluOpType.is_ge,
                            fill=0.0,
                            base=0,
                            pattern=[[1, 128]],
                            channel_multiplier=-1,
                        )
                # AV (+denominator via the ones column of v_aug)
                oa = ovp.tile([128, 512], F32, name="oa", tag="oa")
                ob = ovp.tile([128, 512], F32, name="ob", tag="ob")
                for b in range(nb):
                    st = dict(start=(b == 0), stop=(b == nb - 1))
                    nc.tensor.matmul(
                        oa[0:65, :], va[:, b, 0:65],
                        pa[:, b * 512 : (b + 1) * 512], **st,
                    )
                    nc.tensor.matmul(
                        ob[0:65, :], va[:, b, 65:130],
                        pb[:, b * 512 : (b + 1) * 512], **st,
                    )
                # stage attn rows + denominators -> A2A bounce buffer.
                # dest shard for (batch bb, l-block tau) is 4*bb + tau;
                # row inside shard = 64*j (+256..259 for denoms).
                st1 = stp.tile([128, 512], F32, name="st1", tag="st1")
                nc.scalar.copy(st1[0:64, :], oa[0:64, :])
                nc.scalar.copy(st1[64:128, :], ob[0:64, :])
                for bb, half in ((0, st1[0:64, :]), (1, st1[64:128, :])):
                    sh = SH * (4 * bb + tau)
                    nc.sync.dma_start(
                        bin_[sh + 64 * j : sh + 64 * (j + 1), :], half
                    )
                ds = stp.tile([128, 1024], F32, name="ds", tag="ds")
                nc.vector.tensor_copy(ds[64:65, 0:512], oa[64:65, :])
                nc.vector.tensor_copy(ds[64:65, 512:1024], ob[64:65, :])
                for bb in range(2):
                    sh = SH * (4 * bb + tau)
                    nc.sync.dma_start(
                        bin_[sh + QF + j : sh + QF + j + 1, :],
                        ds[64:65, 512 * bb : 512 * bb + 512],
                    )


def _phase4_oproj(nc, tc, bout, woT, rdram, y):
    """Normalize (divide by softmax denominators) and run o_proj for this
    core's 512 sequence rows against the full Wo."""
    with (
        tc.tile_pool(name="an", bufs=1) as anp,
        tc.tile_pool(name="wo", bufs=2) as wop,
        tc.tile_pool(name="den", bufs=1) as denp,
        tc.tile_pool(name="ysb", bufs=2) as yp,
        tc.tile_pool(name="p4y", bufs=4, space="PSUM") as eyp,
    ):
        # denominators: shard c rows 256:260 = heads 4c..4c+3
        dall = denp.tile([32, 512], F32, name="dall")
        for c in range(NCORES):
            nc.sync.dma_start(
                dall[4 * c : 4 * (c + 1), :],
                bout[SH * c + QF : SH * c + QF + NH_L, :],
            )
        rall_f = denp.tile([32, 512], F32, name="rall_f")
        nc.vector.reciprocal(rall_f[:, :], dall[:, :])
        nc.sync.dma_start(rdram[:, :], rall_f[:, :])

        ans = []
        for ft in range(16):
            c, half = divmod(ft, 2)
            au = anp.tile([128, 512], F32, name=f"au{ft}", tag=f"au{ft}")
            nc.sync.dma_start(
                au[:, :],
                bout[SH * c + 128 * half : SH * c + 128 * (half + 1), :],
            )
            hA = 2 * ft
            hB = 2 * ft + 1
            dv = anp.tile([128, 512], F32, name="dv", tag="dv", bufs=2)
            nc.sync.dma_start(
                dv[0:64, :], rdram[hA : hA + 1, :].partition_broadcast(64)
            )
            nc.sync.dma_start(
                dv[64:128, :], rdram[hB : hB + 1, :].partition_broadcast(64)
            )
            an = anp.tile([128, 512], F32R, name=f"an{ft}", tag=f"an{ft}")
            nc.vector.tensor_mul(an[:, :], au[:, :], dv[:, :])
            ans.append(an)

        for dc in range(4):
            wo_t = wop.tile([128, 16 * 512], F32R, name="wo_t", tag="wo")
            nc.gpsimd.dma_start(
                wo_t.rearrange("p (b d) -> p b d", d=512),
                woT[:, dc * 512 : (dc + 1) * 512].rearrange("(b p) d -> p b d", p=128),
            )
            for m in range(4):
                yps = eyp.tile([128, 512], F32, name="yps", tag="yps")
                for k in range(16):
                    _mmr(
                        nc, yps[:, :],
                        ans[k][:, m * 128 : (m + 1) * 128],
                        wo_t[:, k * 512 : (k + 1) * 512],
                        start=(k == 0), stop=(k == 15),
                    )
                ysb = yp.tile([128, 512], F32, name="ysb", tag="ysb")
                nc.scalar.copy(ysb[:, :], yps[:, :])
                nc.sync.dma_start(
                    y[m * 128 : (m + 1) * 128, dc * 512 : (dc + 1) * 512], ysb[:, :]
                )


def _get_nc():
    if "nc" not in _CACHE:
        _CACHE["nc"] = _build_nc()
    return _CACHE["nc"]


LAST_EXEC_NS = None


def kernel(x, Wq, Wk, Wv, Wo):
    global LAST_EXEC_NS
    x = np.asarray(x, dtype=np.float32)
    Wq = np.asarray(Wq, dtype=np.float32)
    Wk = np.asarray(Wk, dtype=np.float32)
    Wv = np.asarray(Wv, dtype=np.float32)
    Wo = np.asarray(Wo, dtype=np.float32)

    xT0 = np.ascontiguousarray(x[0].T)
    xT1 = np.ascontiguousarray(x[1].T)
    woT = np.ascontiguousarray(Wo.T)

    in_maps = []
    for c in range(NCORES):
        wqT_c = np.ascontiguousarray((SCALE * Wq[QF * c : QF * (c + 1), :]).T)
        wkT_c = np.ascontiguousarray(Wk[DH * c : DH * (c + 1), :].T)
        wvT_c = np.ascontiguousarray(Wv[DH * c : DH * (c + 1), :].T)
        in_maps.append(
            {
                "xT0": xT0,
                "xT1": xT1,
                "wqT": wqT_c,
                "wkT": wkT_c,
                "wvT": wvT_c,
                "woT": woT,
            }
        )

    nc = _get_nc()
    res = run_bass_kernel_spmd(nc, in_maps, core_ids=list(range(NCORES)))
    LAST_EXEC_NS = getattr(res, "exec_time_ns", None)

    out = np.empty((B, L, D), dtype=np.float32)
    for c in range(NCORES):
        b, g = divmod(c, 4)
        out[b, 512 * g : 512 * (g + 1), :] = res.results[c]["y"]
    return out
